# revision 57
# baseline (speedup 1.0000x reference)
"""IndRNN kernel for 8 Trainium2 NeuronCores.

Math: h_t = relu(xw_t + b + u * h_{t-1}), h_0 = ones.  Output all h_t.

Strategy (v3: exact two-scan formulation)
-----------------------------------------
Closed form of the relu recurrence (exact, no approximation):

    p_t = u * p_{t-1} + xw_t,          p_0 = 0        (affine scan)
    q_t = min(u * q_{t-1}, p_t),       q_0 = -h_0     (min scan)
    h_t = relu(p_t - u * q_{t-1})

(Unrolling h_t = max(0, max_s sum_{i=s..t} u^{t-i} xw_i, u^t h_0 + ...);
each inner term is p_t - u^{t-s} p_s, and the q scan tracks the running
min of those suffixes.)  All quantities are O(max|xw|/(1-u)): no
overflow, valid for every u in (0,1) -- no sorting or lane splitting.

Mapping (per core: 4 batch rows x 2 hidden halves = 8 tiles [128, T]):
  - Tensor : xw = W^T x^T in PSUM per 512-col chunk (bf16).
  - Scalar : PSUM -> SBUF bf16 copy with per-lane bias (Identity+bias);
             final relu(-s) with scale=-1 -> bf16.
  - Vector : the two scans; s = u*q_{t-1} - p via bf16 tensor_tensor
             (2x mode).  The second scan emits u*q_t directly using
             state' = u*min(p_t, state) with initial -u.
  - First-batch scans are chunk-chained so the pipeline ramps at matmul
    rate instead of waiting for the full [128, 4096] xw tile.
  - All I/O in bf16 (halves HBM traffic); rel-err ~4e-3 << 2e-2 gate.
"""

import sys

for _p in ("/opt/trn_rl_repo",):
    if _p not in sys.path:
        sys.path.insert(0, _p)

from contextlib import ExitStack

import numpy as np
import ml_dtypes

import concourse.bass as bass
import concourse.tile as tile
from concourse import bacc, mybir
from concourse.bass_utils import run_bass_kernel_spmd

F32 = mybir.dt.float32
BF16 = mybir.dt.bfloat16
ALU = mybir.AluOpType
ACTF = mybir.ActivationFunctionType

B, T, D, H = 32, 4096, 256, 256
NCORES = 8
BLOC = B // NCORES  # batch rows per core
CHUNK = 512         # matmul N-tile (one PSUM bank)


def _build(nc):
    xt_d = nc.declare_dram_parameter("xt", [BLOC, D, T], BF16, isOutput=False)
    w_d = nc.declare_dram_parameter("w", [D, H], BF16, isOutput=False)
    cols_d = nc.declare_dram_parameter("cols", [H, 3], F32, isOutput=False)
    eye_d = nc.declare_dram_parameter("eye", [128, 128], BF16, isOutput=False)
    ney_d = nc.declare_dram_parameter("neye", [128, 128], BF16, isOutput=False)
    out_d = nc.declare_dram_parameter("out", [BLOC, H, T], BF16, isOutput=True)

    nchunks = T // CHUNK

    with tile.TileContext(nc) as tc, ExitStack() as ctx:
        const = ctx.enter_context(tc.tile_pool(name="const", bufs=1))
        xt_pool = ctx.enter_context(tc.tile_pool(name="xt", bufs=2))
        psum_pool = ctx.enter_context(
            tc.tile_pool(name="psum", bufs=4, space=bass.MemorySpace.PSUM)
        )
        psub_pool = ctx.enter_context(
            tc.tile_pool(name="psub", bufs=4, space=bass.MemorySpace.PSUM)
        )
        xw_pool = ctx.enter_context(tc.tile_pool(name="xw", bufs=5))
        p_pool = ctx.enter_context(tc.tile_pool(name="p", bufs=4))
        q_pool = ctx.enter_context(tc.tile_pool(name="q", bufs=4))
        h_pool = ctx.enter_context(tc.tile_pool(name="h", bufs=4))

        # persistent weights / tables
        w_sb = []
        for dh in range(2):
            wt = const.tile([128, H], BF16, tag=f"w{dh}")
            nc.sync.dma_start(wt[:, :], w_d[dh * 128 : (dh + 1) * 128, :])
            w_sb.append(wt)
        bcol_sb, ucol_sb, nucol_sb = [], [], []
        for hh in range(2):
            ct = const.tile([128, 3], F32, tag=f"cols{hh}")
            # scalar ring: lands in parallel with the w tiles on sync
            nc.scalar.dma_start(ct[:, :], cols_d[hh * 128 : (hh + 1) * 128, :])
            # unpack into standalone aligned columns (offset-4 sub-views of
            # the packed tile slow the scan's src0 fetch)
            bt = const.tile([128, 1], F32, tag=f"b{hh}")
            nc.vector.tensor_copy(bt[:, :], ct[:, 0:1])
            ut = const.tile([128, 1], F32, tag=f"u{hh}")
            nc.vector.tensor_copy(ut[:, :], ct[:, 1:2])
            nt = const.tile([128, 1], F32, tag=f"nu{hh}")
            nc.vector.tensor_copy(nt[:, :], ct[:, 2:3])
            bcol_sb.append(bt[:, :])
            ucol_sb.append(ut[:, :])
            nucol_sb.append(nt[:, :])
        eye_sb = const.tile([128, 128], BF16, tag="eye")
        neye_sb = const.tile([128, 128], BF16, tag="neye")

        # chunk grid: first batch uses a ladder so the pipeline starts early
        ladder = [128, 128, 256] + [CHUNK] * 7
        def grid(b):
            sizes = ladder if b == 0 else [CHUNK] * nchunks
            out, c0 = [], 0
            for cc in sizes:
                out.append((c0, cc))
                c0 += cc
            return out

        def emit_subtract(p, r, b, hsl):
            # h = relu(p - u*q_{t-1}) via identity matmuls into PSUM
            # (tensor engine), relu straight out of PSUM (scalar).
            for k in range(nchunks // 2):
                h = h_pool.tile([128, 2 * CHUNK], BF16, tag="h")
                for j in range(2):
                    k0 = (2 * k + j) * CHUNK
                    psb = psub_pool.tile([128, CHUNK], F32, tag="psb")
                    nc.tensor.matmul(
                        psb[:, :], eye_sb[:, :], p[:, k0 : k0 + CHUNK],
                        start=True, stop=False,
                    )
                    nc.tensor.matmul(
                        psb[:, :], neye_sb[:, :], r[:, k0 : k0 + CHUNK],
                        start=False, stop=True,
                    )
                    nc.scalar.activation(
                        h[:, j * CHUNK : (j + 1) * CHUNK], psb[:, :], ACTF.Relu
                    )
                nc.sync.dma_start(
                    out_d[b, hsl, 2 * k * CHUNK : 2 * (k + 1) * CHUNK], h[:, :]
                )

        pending = None
        for b in range(BLOC):
            cgrid = grid(b)
            xt0 = xt_pool.tile([128, T], BF16, tag="xt0")
            xt1 = xt_pool.tile([128, T], BF16, tag="xt1")
            xts = [xt0, xt1]
            if b == 0:
                # chunked, dh-interleaved loads so the first matmul starts early
                for c0, cc in cgrid:
                    for dh in range(2):
                        nc.sync.dma_start(
                            xts[dh][:, c0 : c0 + cc],
                            xt_d[b, dh * 128 : (dh + 1) * 128, c0 : c0 + cc],
                        )
            else:
                for dh in range(2):
                    nc.sync.dma_start(
                        xts[dh][:, :], xt_d[b, dh * 128 : (dh + 1) * 128, :]
                    )
            if b == 0:
                # sync ring after the hot-path loads; needed only ~30us in
                nc.sync.dma_start(eye_sb[:, :], eye_d[:, :])
                nc.sync.dma_start(neye_sb[:, :], ney_d[:, :])
            for hh in range(2):
                hsl = slice(hh * 128, (hh + 1) * 128)
                u_bc = ucol_sb[hh].broadcast_to([128, T])
                

                xwb = xw_pool.tile([128, T], BF16, tag="xw")
                for c0, cc in cgrid:
                    ps = psum_pool.tile([128, cc], F32, tag="ps")
                    for dh in range(2):
                        nc.tensor.matmul(
                            ps[:, :],
                            w_sb[dh][:, hsl],
                            xts[dh][:, c0 : c0 + cc],
                            start=(dh == 0),
                            stop=(dh == 1),
                        )
                    nc.scalar.activation(
                        xwb[:, c0 : c0 + cc], ps[:, :], ACTF.Identity,
                        bias=bcol_sb[hh],
                    )

                last = b == BLOC - 1 and hh == 1
                p = p_pool.tile([128, T], BF16, tag="p")
                r = q_pool.tile([128, T + 1], BF16, tag="r")
                nc.scalar.activation(r[:, 0:1], nucol_sb[hh], ACTF.Copy)
                if last:
                    # chunk scans AND the combine so the tail drains early
                    if pending is not None:
                        emit_subtract(*pending)
                        pending = None
                    tail_grid = [(0, 1024), (1024, 1024), (2048, 1024),
                                 (3072, 512), (3584, 512)]
                    for c0, LC in tail_grid:
                        u_bc_lc = ucol_sb[hh].broadcast_to([128, LC])
                        nc.vector.tensor_tensor_scan(
                            p[:, c0 : c0 + LC], u_bc_lc,
                            xwb[:, c0 : c0 + LC],
                            0.0 if c0 == 0 else p[:, c0 - 1 : c0],
                            op0=ALU.mult, op1=ALU.add,
                        )
                        nc.vector.tensor_tensor_scan(
                            r[:, c0 + 1 : c0 + LC + 1],
                            p[:, c0 : c0 + LC], u_bc_lc,
                            nucol_sb[hh] if c0 == 0 else r[:, c0 : c0 + 1],
                            op0=ALU.min, op1=ALU.mult,
                        )
                        s = h_pool.tile([128, LC], BF16, tag="sl")
                        nc.vector.tensor_tensor(
                            s[:, :], r[:, c0 : c0 + LC],
                            p[:, c0 : c0 + LC], op=ALU.subtract,
                        )
                        h = h_pool.tile([128, LC], BF16, tag="hl")
                        nc.scalar.activation(
                            h[:, :], s[:, :], ACTF.Relu, scale=-1.0
                        )
                        nc.sync.dma_start(
                            out_d[b, hsl, c0 : c0 + LC], h[:, :]
                        )
                    continue
                if b == 0 and hh == 0:
                    # chunk-chained scans: ramp at matmul rate
                    for c0, cc in cgrid:
                        u_bc_cc = ucol_sb[hh].broadcast_to([128, cc])
                        nc.vector.tensor_tensor_scan(
                            p[:, c0 : c0 + cc], u_bc_cc,
                            xwb[:, c0 : c0 + cc],
                            0.0 if c0 == 0 else p[:, c0 - 1 : c0],
                            op0=ALU.mult, op1=ALU.add,
                        )
                        nc.vector.tensor_tensor_scan(
                            r[:, c0 + 1 : c0 + cc + 1],
                            p[:, c0 : c0 + cc], u_bc_cc,
                            nucol_sb[hh] if c0 == 0 else r[:, c0 : c0 + 1],
                            op0=ALU.min, op1=ALU.mult,
                        )
                else:
                    # p_t = u p_{t-1} + xw_t   (p_0 = 0)
                    nc.vector.tensor_tensor_scan(
                        p[:, :], u_bc, xwb[:, :], 0.0, op0=ALU.mult, op1=ALU.add
                    )
                    # r col j = u*q_j via state' = u*min(p_t, state)
                    nc.vector.tensor_tensor_scan(
                        r[:, 1 : T + 1], p[:, :], u_bc, nucol_sb[hh],
                        op0=ALU.min, op1=ALU.mult,
                    )
                if pending is not None:
                    emit_subtract(*pending)
                # defer the identity-matmul subtract one tile so the
                # in-order tensor engine never blocks the next tile's
                # xw matmuls on this tile's scans
                pending = (p, r, b, hsl)


def _host_prep(x, W, b, u):
    x = np.asarray(x, np.float32)
    W = np.asarray(W, np.float32)
    b = np.asarray(b, np.float32)
    u = np.asarray(u, np.float32)

    xt = np.ascontiguousarray(
        np.swapaxes(x, 1, 2).astype(ml_dtypes.bfloat16)
    )  # [B, D, T] bf16
    common = {
        "w": np.ascontiguousarray(W.astype(ml_dtypes.bfloat16)),
        "cols": np.ascontiguousarray(np.stack([b, u, -u], axis=1)),
        "eye": np.eye(128, dtype=ml_dtypes.bfloat16),
        "neye": (-np.eye(128)).astype(ml_dtypes.bfloat16),
    }
    in_maps = []
    for c in range(NCORES):
        m = dict(common)
        m["xt"] = np.ascontiguousarray(xt[c * BLOC : (c + 1) * BLOC])
        in_maps.append(m)
    return in_maps


# set by test harnesses to profile: kernel() stores the raw results here
LAST_RESULT = None


def kernel(x, W, b, u):
    global LAST_RESULT
    import os

    in_maps = _host_prep(x, W, b, u)

    nc = bacc.Bacc("TRN2", target_bir_lowering=False, debug=False)
    _build(nc)
    nc.compile()

    trace = bool(os.environ.get("INDRNN_TRACE"))
    res = run_bass_kernel_spmd(
        nc, in_maps, core_ids=list(range(NCORES)), trace=trace
    )
    LAST_RESULT = res
    out_dev = np.concatenate(
        [np.asarray(r["out"]) for r in res.results], axis=0
    )  # [B, H, T] bf16

    out = np.ascontiguousarray(
        np.swapaxes(out_dev, 1, 2).astype(np.float32)
    )  # [B, T, H] fp32
    return out


# revision 58
# speedup vs baseline: 1.0116x; 1.0116x over previous
"""IndRNN kernel for 8 Trainium2 NeuronCores.

Math: h_t = relu(xw_t + b + u * h_{t-1}), h_0 = ones.  Output all h_t.

Strategy (v3: exact two-scan formulation)
-----------------------------------------
Closed form of the relu recurrence (exact, no approximation):

    p_t = u * p_{t-1} + xw_t,          p_0 = 0        (affine scan)
    q_t = min(u * q_{t-1}, p_t),       q_0 = -h_0     (min scan)
    h_t = relu(p_t - u * q_{t-1})

(Unrolling h_t = max(0, max_s sum_{i=s..t} u^{t-i} xw_i, u^t h_0 + ...);
each inner term is p_t - u^{t-s} p_s, and the q scan tracks the running
min of those suffixes.)  All quantities are O(max|xw|/(1-u)): no
overflow, valid for every u in (0,1) -- no sorting or lane splitting.

Mapping (per core: 4 batch rows x 2 hidden halves = 8 tiles [128, T]):
  - Tensor : xw = W^T x^T in PSUM per 512-col chunk (bf16).
  - Scalar : PSUM -> SBUF bf16 copy with per-lane bias (Identity+bias);
             final relu(-s) with scale=-1 -> bf16.
  - Vector : the two scans; s = u*q_{t-1} - p via bf16 tensor_tensor
             (2x mode).  The second scan emits u*q_t directly using
             state' = u*min(p_t, state) with initial -u.
  - First-batch scans are chunk-chained so the pipeline ramps at matmul
    rate instead of waiting for the full [128, 4096] xw tile.
  - All I/O in bf16 (halves HBM traffic); rel-err ~4e-3 << 2e-2 gate.
"""

import sys

for _p in ("/opt/trn_rl_repo",):
    if _p not in sys.path:
        sys.path.insert(0, _p)

from contextlib import ExitStack

import numpy as np
import ml_dtypes

import concourse.bass as bass
import concourse.tile as tile
from concourse import bacc, mybir
from concourse.bass_utils import run_bass_kernel_spmd

F32 = mybir.dt.float32
BF16 = mybir.dt.bfloat16
ALU = mybir.AluOpType
ACTF = mybir.ActivationFunctionType

B, T, D, H = 32, 4096, 256, 256
NCORES = 8
BLOC = B // NCORES  # batch rows per core
CHUNK = 512         # matmul N-tile (one PSUM bank)


def _build(nc):
    xt_d = nc.declare_dram_parameter("xt", [BLOC, D, T], BF16, isOutput=False)
    w_d = nc.declare_dram_parameter("w", [D, H], BF16, isOutput=False)
    cols_d = nc.declare_dram_parameter("cols", [H, 3], F32, isOutput=False)
    eye_d = nc.declare_dram_parameter("eye", [128, 128], BF16, isOutput=False)
    ney_d = nc.declare_dram_parameter("neye", [128, 128], BF16, isOutput=False)
    out_d = nc.declare_dram_parameter("out", [BLOC, H, T], BF16, isOutput=True)

    nchunks = T // CHUNK

    with tile.TileContext(nc) as tc, ExitStack() as ctx:
        const = ctx.enter_context(tc.tile_pool(name="const", bufs=1))
        xt_pool = ctx.enter_context(tc.tile_pool(name="xt", bufs=2))
        psum_pool = ctx.enter_context(
            tc.tile_pool(name="psum", bufs=4, space=bass.MemorySpace.PSUM)
        )
        psub_pool = ctx.enter_context(
            tc.tile_pool(name="psub", bufs=4, space=bass.MemorySpace.PSUM)
        )
        xw_pool = ctx.enter_context(tc.tile_pool(name="xw", bufs=5))
        p_pool = ctx.enter_context(tc.tile_pool(name="p", bufs=4))
        q_pool = ctx.enter_context(tc.tile_pool(name="q", bufs=4))
        h_pool = ctx.enter_context(tc.tile_pool(name="h", bufs=4))

        # persistent weights / tables
        w_sb = []
        for dh in range(2):
            wt = const.tile([128, H], BF16, tag=f"w{dh}")
            nc.sync.dma_start(wt[:, :], w_d[dh * 128 : (dh + 1) * 128, :])
            w_sb.append(wt)
        bcol_sb, ucol_sb, nucol_sb = [], [], []
        for hh in range(2):
            ct = const.tile([128, 3], F32, tag=f"cols{hh}")
            # scalar ring: lands in parallel with the w tiles on sync
            nc.scalar.dma_start(ct[:, :], cols_d[hh * 128 : (hh + 1) * 128, :])
            # unpack into standalone aligned columns (offset-4 sub-views of
            # the packed tile slow the scan's src0 fetch)
            bt = const.tile([128, 1], F32, tag=f"b{hh}")
            nc.vector.tensor_copy(bt[:, :], ct[:, 0:1])
            ut = const.tile([128, 1], F32, tag=f"u{hh}")
            nc.vector.tensor_copy(ut[:, :], ct[:, 1:2])
            nt = const.tile([128, 1], F32, tag=f"nu{hh}")
            nc.vector.tensor_copy(nt[:, :], ct[:, 2:3])
            bcol_sb.append(bt[:, :])
            ucol_sb.append(ut[:, :])
            nucol_sb.append(nt[:, :])
        eye_sb = const.tile([128, 128], BF16, tag="eye")
        neye_sb = const.tile([128, 128], BF16, tag="neye")

        # chunk grid: first batch uses a ladder so the pipeline starts early
        ladder = [128, 128, 256] + [CHUNK] * 7
        def grid(b):
            sizes = ladder if b == 0 else [CHUNK] * nchunks
            out, c0 = [], 0
            for cc in sizes:
                out.append((c0, cc))
                c0 += cc
            return out

        def emit_subtract(p, r, b, hsl):
            # h = relu(p - u*q_{t-1}) via identity matmuls into PSUM
            # (tensor engine), relu straight out of PSUM (scalar).
            for k in range(nchunks // 2):
                h = h_pool.tile([128, 2 * CHUNK], BF16, tag="h")
                for j in range(2):
                    k0 = (2 * k + j) * CHUNK
                    psb = psub_pool.tile([128, CHUNK], F32, tag="psb")
                    nc.tensor.matmul(
                        psb[:, :], eye_sb[:, :], p[:, k0 : k0 + CHUNK],
                        start=True, stop=False,
                    )
                    nc.tensor.matmul(
                        psb[:, :], neye_sb[:, :], r[:, k0 : k0 + CHUNK],
                        start=False, stop=True,
                    )
                    nc.scalar.activation(
                        h[:, j * CHUNK : (j + 1) * CHUNK], psb[:, :], ACTF.Relu
                    )
                nc.sync.dma_start(
                    out_d[b, hsl, 2 * k * CHUNK : 2 * (k + 1) * CHUNK], h[:, :]
                )

        pending = None
        for b in range(BLOC):
            cgrid = grid(b)
            xt0 = xt_pool.tile([128, T], BF16, tag="xt0")
            xt1 = xt_pool.tile([128, T], BF16, tag="xt1")
            xts = [xt0, xt1]
            if b == 0:
                # chunked, dh-interleaved loads so the first matmul starts early
                for c0, cc in cgrid:
                    for dh in range(2):
                        nc.sync.dma_start(
                            xts[dh][:, c0 : c0 + cc],
                            xt_d[b, dh * 128 : (dh + 1) * 128, c0 : c0 + cc],
                        )
            else:
                for dh in range(2):
                    nc.sync.dma_start(
                        xts[dh][:, :], xt_d[b, dh * 128 : (dh + 1) * 128, :]
                    )
            if b == 0:
                # sync ring after the hot-path loads; needed only ~30us in
                nc.sync.dma_start(eye_sb[:, :], eye_d[:, :])
                nc.sync.dma_start(neye_sb[:, :], ney_d[:, :])
            for hh in range(2):
                hsl = slice(hh * 128, (hh + 1) * 128)
                u_bc = ucol_sb[hh].broadcast_to([128, T])
                

                xwb = xw_pool.tile([128, T], BF16, tag="xw")
                for c0, cc in cgrid:
                    ps = psum_pool.tile([128, cc], F32, tag="ps")
                    for dh in range(2):
                        nc.tensor.matmul(
                            ps[:, :],
                            w_sb[dh][:, hsl],
                            xts[dh][:, c0 : c0 + cc],
                            start=(dh == 0),
                            stop=(dh == 1),
                        )
                    nc.scalar.activation(
                        xwb[:, c0 : c0 + cc], ps[:, :], ACTF.Identity,
                        bias=bcol_sb[hh],
                    )

                last = b == BLOC - 1 and hh == 1
                p = p_pool.tile([128, T], BF16, tag="p")
                r = q_pool.tile([128, T + 1], BF16, tag="r")
                nc.vector.tensor_copy(r[:, 0:1], nucol_sb[hh])
                if last:
                    # chunk scans AND the combine so the tail drains early
                    if pending is not None:
                        emit_subtract(*pending)
                        pending = None
                    tail_grid = [(0, 1024), (1024, 1024), (2048, 1024),
                                 (3072, 512), (3584, 512)]
                    for c0, LC in tail_grid:
                        u_bc_lc = ucol_sb[hh].broadcast_to([128, LC])
                        nc.vector.tensor_tensor_scan(
                            p[:, c0 : c0 + LC], u_bc_lc,
                            xwb[:, c0 : c0 + LC],
                            0.0 if c0 == 0 else p[:, c0 - 1 : c0],
                            op0=ALU.mult, op1=ALU.add,
                        )
                        nc.vector.tensor_tensor_scan(
                            r[:, c0 + 1 : c0 + LC + 1],
                            p[:, c0 : c0 + LC], u_bc_lc,
                            nucol_sb[hh] if c0 == 0 else r[:, c0 : c0 + 1],
                            op0=ALU.min, op1=ALU.mult,
                        )
                        s = h_pool.tile([128, LC], BF16, tag="sl")
                        nc.vector.tensor_tensor(
                            s[:, :], r[:, c0 : c0 + LC],
                            p[:, c0 : c0 + LC], op=ALU.subtract,
                        )
                        h = h_pool.tile([128, LC], BF16, tag="hl")
                        nc.scalar.activation(
                            h[:, :], s[:, :], ACTF.Relu, scale=-1.0
                        )
                        nc.sync.dma_start(
                            out_d[b, hsl, c0 : c0 + LC], h[:, :]
                        )
                    continue
                if b == 0 and hh == 0:
                    # chunk-chained scans: ramp at matmul rate
                    for c0, cc in cgrid:
                        u_bc_cc = ucol_sb[hh].broadcast_to([128, cc])
                        nc.vector.tensor_tensor_scan(
                            p[:, c0 : c0 + cc], u_bc_cc,
                            xwb[:, c0 : c0 + cc],
                            0.0 if c0 == 0 else p[:, c0 - 1 : c0],
                            op0=ALU.mult, op1=ALU.add,
                        )
                        nc.vector.tensor_tensor_scan(
                            r[:, c0 + 1 : c0 + cc + 1],
                            p[:, c0 : c0 + cc], u_bc_cc,
                            nucol_sb[hh] if c0 == 0 else r[:, c0 : c0 + 1],
                            op0=ALU.min, op1=ALU.mult,
                        )
                else:
                    # p_t = u p_{t-1} + xw_t   (p_0 = 0)
                    nc.vector.tensor_tensor_scan(
                        p[:, :], u_bc, xwb[:, :], 0.0, op0=ALU.mult, op1=ALU.add
                    )
                    # r col j = u*q_j via state' = u*min(p_t, state)
                    nc.vector.tensor_tensor_scan(
                        r[:, 1 : T + 1], p[:, :], u_bc, nucol_sb[hh],
                        op0=ALU.min, op1=ALU.mult,
                    )
                if pending is not None:
                    emit_subtract(*pending)
                # defer the identity-matmul subtract one tile so the
                # in-order tensor engine never blocks the next tile's
                # xw matmuls on this tile's scans
                pending = (p, r, b, hsl)


def _host_prep(x, W, b, u):
    x = np.asarray(x, np.float32)
    W = np.asarray(W, np.float32)
    b = np.asarray(b, np.float32)
    u = np.asarray(u, np.float32)

    xt = np.ascontiguousarray(
        np.swapaxes(x, 1, 2).astype(ml_dtypes.bfloat16)
    )  # [B, D, T] bf16
    common = {
        "w": np.ascontiguousarray(W.astype(ml_dtypes.bfloat16)),
        "cols": np.ascontiguousarray(np.stack([b, u, -u], axis=1)),
        "eye": np.eye(128, dtype=ml_dtypes.bfloat16),
        "neye": (-np.eye(128)).astype(ml_dtypes.bfloat16),
    }
    in_maps = []
    for c in range(NCORES):
        m = dict(common)
        m["xt"] = np.ascontiguousarray(xt[c * BLOC : (c + 1) * BLOC])
        in_maps.append(m)
    return in_maps


# set by test harnesses to profile: kernel() stores the raw results here
LAST_RESULT = None


def kernel(x, W, b, u):
    global LAST_RESULT
    import os

    in_maps = _host_prep(x, W, b, u)

    nc = bacc.Bacc("TRN2", target_bir_lowering=False, debug=False)
    _build(nc)
    nc.compile()

    trace = bool(os.environ.get("INDRNN_TRACE"))
    res = run_bass_kernel_spmd(
        nc, in_maps, core_ids=list(range(NCORES)), trace=trace
    )
    LAST_RESULT = res
    out_dev = np.concatenate(
        [np.asarray(r["out"]) for r in res.results], axis=0
    )  # [B, H, T] bf16

    out = np.ascontiguousarray(
        np.swapaxes(out_dev, 1, 2).astype(np.float32)
    )  # [B, T, H] fp32
    return out
